# revision 17
# baseline (speedup 1.0000x reference)
"""Trainium2 Bass kernel for relative-position attention (nn_Attention_14714557956326).

Full inputs:
  x       [4, 1024, 1024] f32
  Wq      [1024, 1024]    f32   (dim -> 16 heads * 64)
  Wkv     [1024, 2048]    f32   (k cols 0..1023, v cols 1024..2047)
  pos_emb [1025, 64]      f32
Output: [4, 16, 1024, 64] f32  (softmax(q k^T * s + rel_pos_bias) v, per head)

Sharding: 8 cores; core c handles batch c//2, heads 8*(c%2) .. +8.

v3 design (vs v2 baseline):
- S psum tiles are [128, 1024] (2 banks, matmuls write 512-halves), so exp is
  ONE wide activation per head-tile; its accum_out produces the softmax
  denominator Z for free (no ones-column in V, no Z row in the PV output).
- QP (pos-table) matmuls and ext copies are trimmed to the column range the
  skew actually reads (81% of the naive width); ext window is 1536 wide with
  a group-independent diagonal AP.
- VQ (the g=-512 clamp value q.T0) is computed on the host and shipped as a
  tiny input instead of a PE projection.
- PV packs the head pair into one [128,512] psum bank via tile_position
  (head0 -> out partitions 0-63, head1 -> 64-127).
- Both heads of a tile share one [128, 2048] exp-output tile -> ONE batched
  transpose per (pair, tile) into a pair-combined P^T buffer.
- Projections / QP groups / PV are emitted as small "filler" units interleaved
  into the attention tile stream so the PE never has a long dependency stall
  and runs warm (HAM clock at 2.4 GHz).
"""

import sys

sys.path.insert(0, "/opt/trn_rl_repo")

import numpy as np
from collections import deque
from contextlib import ExitStack

import concourse.bass as bass
import concourse.bacc as bacc
import concourse.tile as tile
from concourse import mybir
from concourse.ap import AP
from concourse.bass_utils import run_bass_kernel_spmd

# ---------------- problem constants ----------------
B = 4
N_HEADS_TOT = 16
D = 64
DIM = 1024
SEQ = 1024
MAX_POS = 512
TABLE = 2 * MAX_POS + 1  # 1025
SCALE = D ** -0.5

NC = 8              # cores
NH = 8              # heads per core
HD = NH * D         # 512 projected cols per matrix per core
KC = DIM // 128     # 8 contraction chunks
RW = 1024           # table cols c=0..1023; c=1024 (dist -512) via host VQ
EXTW = 1536         # ext window width (read span of one 4-tile skew group)
GB = 4              # row-tiles per skew group
C_SHIFT = 8.0       # exp(x - C) to keep fp16 P in range

F32 = mybir.dt.float32
F16 = mybir.dt.float16

_cached = {}


def _qp_range(t):
    """Table-column range [lo, hi] actually read by the skew for row-tile t."""
    lo = max(0, 385 - 128 * t)
    hi = min(1023, 1535 - 128 * t)
    return lo, hi


def build_nc(seq=SEQ, nh=NH, bench_iters=1, ablate=()):
    """Build the per-core Bass program (SPMD: same program on all 8 cores)."""
    it = seq // 128
    hd = nh * D
    npair = nh // 2

    nc = bacc.Bacc(
        "TRN2",
        target_bir_lowering=False,
        debug=False,
        enable_asserts=False,
        num_devices=NC,
    )

    xT = nc.dram_tensor("xT", [DIM, seq], F16, kind="ExternalInput")
    Wq = nc.dram_tensor("Wq", [DIM, hd], F16, kind="ExternalInput")
    Wk = nc.dram_tensor("Wk", [DIM, hd], F16, kind="ExternalInput")
    Wv = nc.dram_tensor("Wv", [DIM, hd], F16, kind="ExternalInput")
    TrevT = nc.dram_tensor("TrevT", [D, RW], F16, kind="ExternalInput")
    VQT = nc.dram_tensor("VQT", [128, it, nh], F16, kind="ExternalInput")
    I128 = nc.dram_tensor("I128", [128, 128], F16, kind="ExternalInput")
    out = nc.dram_tensor("out", [npair, 2, 128, 512], F16, kind="ExternalOutput")
    zout = nc.dram_tensor("zout", [128, npair * 2 * it], F32, kind="ExternalOutput")

    with tile.TileContext(nc) as tc, ExitStack() as ctx:
        if bench_iters > 1:
            ctx.enter_context(
                tc.For_i(
                    0, bench_iters, 1,
                    hint_engines=(
                        mybir.EngineType.PE,
                        mybir.EngineType.DVE,
                        mybir.EngineType.Activation,
                        mybir.EngineType.SP,
                        mybir.EngineType.Pool,
                    ),
                    name="bench",
                )
            )
        # ---------------- persistent pools ----------------
        res = ctx.enter_context(tc.tile_pool(name="res", bufs=1))
        att = ctx.enter_context(tc.tile_pool(name="att", bufs=3))
        ptp_pool = ctx.enter_context(tc.tile_pool(name="ptp", bufs=1))
        ps_big = ctx.enter_context(tc.tile_pool(name="psB", bufs=3, space="PSUM"))
        ps_m = ctx.enter_context(tc.tile_pool(name="psM", bufs=2, space="PSUM"))

        trev_dup = res.tile([128, RW], F16, tag="trevdup")
        ident = res.tile([128, 128], F16, tag="ident")
        exp_bias = res.tile([128, 1], F32, tag="expbias")
        vqt = res.tile([128, it, nh], F16, tag="vqt")
        nc.gpsimd.memset(exp_bias[:], -C_SHIFT)

        x_r = res.tile([128, KC, seq], F16, tag="xr")
        wq_r = res.tile([128, KC, hd], F16, tag="wq")
        wk_r = res.tile([128, KC, hd], F16, tag="wk")
        wv_r = res.tile([128, KC, hd], F16, tag="wv")
        qT = [res.tile([128, seq], F16, tag=f"qT{m}", name=f"qT{m}") for m in range(npair)]
        kT = [res.tile([128, seq], F16, tag=f"kT{m}", name=f"kT{m}") for m in range(npair)]
        V64 = [res.tile([128, hd], F16, tag=f"V{j}", name=f"V{j}") for j in range(it)]
        z = res.tile([128, npair * 2 * it], F32, tag="z")
        # pair-combined P^T, double-buffered by pair parity: [k-part, hh, jb, q]
        ptp = [
            ptp_pool.tile([128, 2, it, seq], F16, tag=f"ptp{par}", name=f"ptp{par}")
            for par in range(min(2, npair))
        ]

        # ---------------- input loads (gating order) ----------------
        nc.scalar.dma_start(wq_r[:], Wq[:].rearrange("(a p) s -> p a s", p=128))
        nc.sync.dma_start(x_r[:], xT[:].rearrange("(a p) s -> p a s", p=128))
        nc.scalar.dma_start(wk_r[:], Wk[:].rearrange("(a p) s -> p a s", p=128))
        nc.sync.dma_start(trev_dup[0:64, :], TrevT[:])
        nc.sync.dma_start(trev_dup[64:128, :], TrevT[:])
        nc.scalar.dma_start(vqt[:], VQT[:])
        nc.sync.dma_start(ident[:], I128[:])
        nc.scalar.dma_start(wv_r[:], Wv[:].rearrange("(a p) s -> p a s", p=128))

        # ---------------- work units ----------------
        def proj_qk_group(m, which, sh):
            """One [128,512] projection group: q or k of pair m, seq half sh."""
            wr = wq_r if which == "q" else wk_r
            dest = qT if which == "q" else kT
            ps = ps_m.tile([128, 512], F32, tag="m512", bufs=2)
            for kc in range(KC):
                nc.tensor.matmul(
                    ps[:],
                    wr[:, kc, 128 * m: 128 * (m + 1)],
                    x_r[:, kc, 512 * sh: 512 * (sh + 1)],
                    start=(kc == 0),
                    stop=(kc == KC - 1),
                )
            nc.vector.tensor_copy(dest[m][:, 512 * sh: 512 * (sh + 1)], ps[:])

        def vproj_group(jt):
            ps = ps_m.tile([128, 512], F32, tag="m512", bufs=2)
            for kc in range(KC):
                nc.tensor.matmul(
                    ps[:, 0:hd],
                    x_r[:, kc, 128 * jt: 128 * (jt + 1)],
                    wv_r[:, kc, :],
                    start=(kc == 0),
                    stop=(kc == KC - 1),
                )
            nc.vector.tensor_copy(V64[jt][:], ps[:, 0:hd])

        exts = {}

        def qp_slice(m, tg, tt, hh):
            """QP matmuls + trimmed ext copy + edge fills for one (tile, head)."""
            t = GB * tg + tt
            h = 2 * m + hh
            lo, hi = _qp_range(t)
            base = 512 if t < GB else 0
            key = (h, tg)
            if key not in exts:
                exts[key] = att.tile(
                    [128, GB, EXTW], F16, tag="ext", name=f"ext{h}_{tg}", bufs=3
                )
            ext = exts[key]
            qps = ps_big.tile([128, 1024], F32, tag="big", bufs=3,
                              name=f"qp{h}_{t}")
            b64 = 64 * hh
            q_t = qT[m][b64: b64 + 64, 128 * t: 128 * (t + 1)]
            t_h = trev_dup[b64: b64 + 64, :]
            nc.tensor.matmul(qps[:, lo:512], q_t, t_h[:, lo:512],
                             start=True, stop=True)
            nc.tensor.matmul(qps[:, 512: hi + 1], q_t, t_h[:, 512: hi + 1],
                             start=True, stop=True)
            nc.vector.tensor_copy(
                ext[:, tt, 511 + lo - base: 512 + hi - base], qps[:, lo: hi + 1]
            )
            if t <= 3:
                nc.gpsimd.tensor_copy(ext[:, tt, 1023:1024], vqt[:, t, h: h + 1])
                rw = 511 - 128 * t
                if rw > 0:
                    nc.gpsimd.tensor_copy(
                        ext[:, tt, 1024: 1024 + rw],
                        ext[:, tt, 1023:1024].to_broadcast([128, rw]),
                    )
            else:
                lw = 128 * t - 385
                if lw > 0:
                    nc.gpsimd.tensor_copy(
                        ext[:, tt, 511 - lw: 511],
                        ext[:, tt, 511:512].to_broadcast([128, lw]),
                    )

        poss = {}

        def qp_skew(m, tg, hh):
            """Batched diagonal skew DMA for one head's 4-tile group (SWDGE)."""
            h = 2 * m + hh
            ext = exts.pop((h, tg))
            pos4 = att.tile([128, GB, seq], F16, tag="pos",
                            name=f"pos{h}_{tg}", bufs=3)
            poss[(h, tg)] = pos4
            extf = ext[:]
            # two half-group DMAs: finer DMA-pool interleaving so transposes
            # queue behind at most ~1.5us of skew transfer
            for half in range(2):
                diag = AP(
                    tensor=extf.tensor,
                    offset=extf.offset + 511 + half * 2 * (EXTW - 128),
                    ap=[[GB * EXTW - 1, 128], [EXTW - 128, 2], [1, seq]],
                )
                nc.gpsimd.dma_start(pos4[:, 2 * half: 2 * half + 2, :], diag)

        def emit_tile(m, t, pop_filler):
            """QK + pos-add + exp + transpose for both heads of (pair m, tile t)."""
            S = {}
            for hh in range(2):
                b64 = 64 * hh
                S[hh] = ps_big.tile([128, 1024], F32, tag="big", bufs=3,
                                    name=f"s{2*m+hh}_{t}")
                for half in range(2):
                    sl = slice(512 * half, 512 * (half + 1))
                    nc.tensor.matmul(
                        S[hh][:, sl],
                        qT[m][b64: b64 + 64, 128 * t: 128 * (t + 1)],
                        kT[m][b64: b64 + 64, sl],
                        start=True,
                        stop=False,
                    )
                pop_filler()
            for hh in range(2):
                h = 2 * m + hh
                pos4 = poss[(h, t // GB)]
                for half in range(2):
                    sl = slice(512 * half, 512 * (half + 1))
                    nc.tensor.matmul(
                        S[hh][:, sl],
                        ident[:],
                        pos4[:, t % GB, sl],
                        start=False,
                        stop=True,
                    )
                if t % GB == GB - 1:
                    poss.pop((h, t // GB), None)
            pop_filler()
            pt = att.tile([128, 2 * seq], F16, tag="pt", name=f"p{m}_{t}", bufs=4)
            for hh in range(2):
                zc = 16 * m + 8 * hh + t
                nc.scalar.activation(
                    pt[:, seq * hh: seq * (hh + 1)], S[hh][:],
                    mybir.ActivationFunctionType.Exp,
                    bias=exp_bias[:], scale=1.0,
                    accum_out=z[:, zc: zc + 1],
                )
            nc.sync.dma_start_transpose(
                out=ptp[m % 2][:, :, :, 128 * t: 128 * (t + 1)], in_=pt[:]
            )
            pop_filler()

        def pv_chunk(m, blk):
            """PV for pair m, q-tile block blk (4 row-tiles): P^T chunks are
            the stationary operand, V the moving one, so the output lands in
            [q, e] orientation at full PE array utilization."""
            ps = ps_m.tile([128, 512], F32, tag="m512", bufs=2, name=f"pv{m}_{blk}")
            for sub in range(4):
                t = 4 * blk + sub
                for jb in range(it):
                    for hh in range(2):
                        h = 2 * m + hh
                        nc.tensor.matmul(
                            ps[:, 128 * sub + 64 * hh: 128 * sub + 64 * hh + 64],
                            ptp[m % 2][:, hh, jb, 128 * t: 128 * (t + 1)],
                            V64[jb][:, 64 * h: 64 * h + 64],
                            # one 2KB PSUM zero-region per bank: only the very
                            # first matmul may start, later ones write into
                            # pending-zero bytes (read-as-0 then accumulate)
                            start=(sub == 0 and jb == 0 and hh == 0),
                            stop=(sub == 3 and jb == it - 1 and hh == 1),
                            skip_group_check=True,
                        )
            ot = att.tile([128, 512], F16, tag="ot", bufs=1, name=f"ot{m}_{blk}")
            nc.scalar.copy(ot[:], ps[:])
            nc.scalar.dma_start(out[m][blk], ot[:])

        # ---------------- emission schedule ----------------
        def qp_units(m, tg):
            u = [(lambda m=m, tg=tg, tt=tt, hh=hh: qp_slice(m, tg, tt, hh))
                 for tt in range(GB) for hh in range(2)]
            u += [(lambda m=m, tg=tg, hh=hh: qp_skew(m, tg, hh)) for hh in range(2)]
            return u

        def proj_units(m):
            return [(lambda m=m, w=w, sh=sh: proj_qk_group(m, w, sh))
                    for w in ("q", "k") for sh in range(seq // 512)]

        if npair == 4 and seq == SEQ:
            # optimized interleave for the real config.
            # prologue: proj pair 0, then qp(0,0) zipped with proj pair 1 + v 0/1
            for u in proj_units(0):
                u()
            A = deque(qp_units(0, 0))
            Bq = deque(proj_units(1) + [lambda: vproj_group(0), lambda: vproj_group(1)])
            while A or Bq:
                if A:
                    A.popleft()()
                if Bq:
                    Bq.popleft()()
            # filler queue popped 4x per tile (pop index = (8m+t)*4 + k).
            # Ordered so every pos4 group lands >=2 tiles before its ident,
            # projections land before their consumers, and PV(m) pops only
            # after pair m's tiles are fully emitted (see need-by analysis).
            nopads = [None] * 8
            fill = deque(
                qp_units(0, 1)                                     # pops 0-9
                + qp_units(1, 0)                                   # 10-19
                + proj_units(2)                                    # 20-23
                + qp_units(1, 1)                                   # 24-33
                + qp_units(2, 0)                                   # 34-43
                + [(lambda jt=jt: vproj_group(jt)) for jt in range(2, it)]  # 44-49
                + [lambda: pv_chunk(0, 0), lambda: pv_chunk(0, 1)]  # 50-51
                + proj_units(3)                                    # 52-55
                + qp_units(2, 1)                                   # 56-65
                + qp_units(3, 0)                                   # 66-75
                + [lambda: pv_chunk(1, 0), lambda: pv_chunk(1, 1)]  # 76-77
                + qp_units(3, 1)                                   # 78-87
                + nopads + nopads                                  # 88-103
                + [lambda: pv_chunk(2, 0), lambda: pv_chunk(2, 1)]  # 104-105
            )

            def pop_filler():
                if fill:
                    u = fill.popleft()
                    if u is not None:
                        u()

            for m in range(npair):
                for t in range(it):
                    emit_tile(m, t, pop_filler)
            while fill:
                u = fill.popleft()
                if u is not None:
                    u()
            pv_chunk(3, 0)
            pv_chunk(3, 1)
        else:
            # simple order for small sim configs (correctness only)
            for m in range(npair):
                for u in proj_units(m):
                    u()
            for jt in range(it):
                vproj_group(jt)
            nop = lambda: None
            for m in range(npair):
                for tg in range(it // GB):
                    for u in qp_units(m, tg):
                        u()
                for t in range(it):
                    emit_tile(m, t, nop)
                pv_chunk(m, 0)
                pv_chunk(m, 1)

        nc.sync.dma_start(zout[:], z[:])

    nc.compile()
    return nc


def prep_inputs(x, Wq, Wkv, pos_emb):
    """Host-side shard prep: returns list of 8 per-core input dicts (fp16)."""
    x = np.asarray(x, dtype=np.float32)
    Wq = np.asarray(Wq, dtype=np.float32)
    Wkv = np.asarray(Wkv, dtype=np.float32)
    pos_emb = np.asarray(pos_emb, dtype=np.float32)

    Wq_s = (Wq * SCALE).astype(np.float32)
    trevT = np.ascontiguousarray(pos_emb[::-1].T[:, :RW])    # [64, 1024], col c = T[1024-c]
    trevT16 = trevT.astype(np.float16)
    ident = np.eye(128, dtype=np.float16)
    it = SEQ // 128

    in_maps = []
    for c in range(NC):
        b, hg = c // 2, c % 2
        hs = slice(HD * hg, HD * (hg + 1))
        wq_c = np.ascontiguousarray(Wq_s[:, hs])
        # VQ: per-head derived value so that vq[i, h] = q_h(i) . T[0] (dist -512)
        wstar = np.einsum(
            "dhe,e->dh", wq_c.reshape(DIM, NH, D), pos_emb[0].astype(np.float32)
        )
        vq = (x[b] @ wstar).astype(np.float16)               # [seq, nh]
        vqt = np.ascontiguousarray(vq.reshape(it, 128, NH).transpose(1, 0, 2))
        in_maps.append(
            {
                "xT": np.ascontiguousarray(x[b].T).astype(np.float16),
                "Wq": wq_c.astype(np.float16),
                "Wk": np.ascontiguousarray(Wkv[:, hs]).astype(np.float16),
                "Wv": np.ascontiguousarray(Wkv[:, DIM:][:, hs]).astype(np.float16),
                "TrevT": trevT16,
                "VQT": vqt,
                "I128": ident,
            }
        )
    return in_maps


def assemble(results):
    """results: list of 8 out maps ({'out': [4,128,1024] f16, 'zout': [128,64] f32})
    -> [4,16,1024,64] f32.  Normalization (divide by Z) happens here."""
    it = SEQ // 128
    out = np.empty((B, N_HEADS_TOT, SEQ, D), dtype=np.float32)
    for c in range(NC):
        b, hg = c // 2, c % 2
        o = np.asarray(results[c]["out"], dtype=np.float32)   # [4, 2, 128, 512]
        zf = np.asarray(results[c]["zout"], dtype=np.float32) # [128, 64]
        z = zf.reshape(128, NH // 2, 2, it)
        # o[m, qb, r, 128*sub + 64*hh + e] = unnorm out(head 2m+hh, i=128*(4qb+sub)+r, e)
        o5 = o.reshape(NH // 2, 2, 128, 4, 2, D)              # [m, qb, r, sub, hh, e]
        for m in range(NH // 2):
            for hh in range(2):
                h = NH * hg + 2 * m + hh
                un = o5[m, :, :, :, hh, :].transpose(0, 2, 1, 3).reshape(SEQ, D)
                zi = z[:, m, hh, :].T.reshape(SEQ)            # [1024]
                out[b, h] = un / zi[:, None]
    return out


def kernel(x, Wq, Wkv, pos_emb, trace=False, trace_kwargs=None, bench_iters=1, ablate=()):
    key = ("nc", bench_iters, tuple(sorted(ablate)))
    if key not in _cached:
        _cached[key] = build_nc(bench_iters=bench_iters, ablate=ablate)
    nc = _cached[key]
    in_maps = prep_inputs(x, Wq, Wkv, pos_emb)
    res = run_bass_kernel_spmd(
        nc, in_maps, list(range(NC)), trace=trace, **(trace_kwargs or {})
    )
    out = assemble(res.results)
    if trace:
        _cached["last_result"] = res
    return out


if __name__ == "__main__":
    rng = np.random.default_rng(0)
    x = rng.standard_normal((B, SEQ, DIM), dtype=np.float32)
    Wq = (rng.standard_normal((DIM, 1024), dtype=np.float32) * DIM ** -0.5)
    Wkv = (rng.standard_normal((DIM, 2048), dtype=np.float32) * DIM ** -0.5)
    pos_emb = rng.standard_normal((TABLE, D), dtype=np.float32)
    out = kernel(x, Wq, Wkv, pos_emb)
    print("out shape", out.shape, "finite:", np.isfinite(out).all())


# revision 21
# speedup vs baseline: 1.1147x; 1.1147x over previous
"""Trainium2 Bass kernel for relative-position attention (nn_Attention_14714557956326).

Full inputs:
  x       [4, 1024, 1024] f32
  Wq      [1024, 1024]    f32   (dim -> 16 heads * 64)
  Wkv     [1024, 2048]    f32   (k cols 0..1023, v cols 1024..2047)
  pos_emb [1025, 64]      f32
Output: [4, 16, 1024, 64] f32  (softmax(q k^T * s + rel_pos_bias) v, per head)

Sharding: 8 cores; core c handles batch c//2, heads 8*(c%2) .. +8.

v3 design (vs v2 baseline):
- S psum tiles are [128, 1024] (2 banks, matmuls write 512-halves), so exp is
  ONE wide activation per head-tile; its accum_out produces the softmax
  denominator Z for free (no ones-column in V, no Z row in the PV output).
- QP (pos-table) matmuls and ext copies are trimmed to the column range the
  skew actually reads (81% of the naive width); ext window is 1536 wide with
  a group-independent diagonal AP.
- VQ (the g=-512 clamp value q.T0) is computed on the host and shipped as a
  tiny input instead of a PE projection.
- PV packs the head pair into one [128,512] psum bank via tile_position
  (head0 -> out partitions 0-63, head1 -> 64-127).
- Both heads of a tile share one [128, 2048] exp-output tile -> ONE batched
  transpose per (pair, tile) into a pair-combined P^T buffer.
- Projections / QP groups / PV are emitted as small "filler" units interleaved
  into the attention tile stream so the PE never has a long dependency stall
  and runs warm (HAM clock at 2.4 GHz).
"""

import sys

sys.path.insert(0, "/opt/trn_rl_repo")

import numpy as np
from collections import deque
from contextlib import ExitStack

import concourse.bass as bass
import concourse.bacc as bacc
import concourse.tile as tile
from concourse import mybir
from concourse.ap import AP
from concourse.bass_utils import run_bass_kernel_spmd

# ---------------- problem constants ----------------
B = 4
N_HEADS_TOT = 16
D = 64
DIM = 1024
SEQ = 1024
MAX_POS = 512
TABLE = 2 * MAX_POS + 1  # 1025
SCALE = D ** -0.5

NC = 8              # cores
NH = 8              # heads per core
HD = NH * D         # 512 projected cols per matrix per core
KC = DIM // 128     # 8 contraction chunks
RW = 1024           # table cols c=0..1023; c=1024 (dist -512) via host VQ
EXTW = 1536         # ext window width (read span of one 4-tile skew group)
GB = 4              # row-tiles per skew group
C_SHIFT = 8.0       # exp(x - C) to keep fp16 P in range

F32 = mybir.dt.float32
F16 = mybir.dt.float16

_cached = {}


def _qp_range(t):
    """Table-column range [lo, hi] actually read by the skew for row-tile t."""
    lo = max(0, 385 - 128 * t)
    hi = min(1023, 1535 - 128 * t)
    return lo, hi


def build_nc(seq=SEQ, nh=NH, bench_iters=1, ablate=()):
    """Build the per-core Bass program (SPMD: same program on all 8 cores)."""
    it = seq // 128
    hd = nh * D
    npair = nh // 2

    nc = bacc.Bacc(
        "TRN2",
        target_bir_lowering=False,
        debug=False,
        enable_asserts=False,
        num_devices=NC,
    )

    xT = nc.dram_tensor("xT", [DIM, seq], F16, kind="ExternalInput")
    Wq = nc.dram_tensor("Wq", [DIM, hd], F16, kind="ExternalInput")
    Wk = nc.dram_tensor("Wk", [DIM, hd], F16, kind="ExternalInput")
    Wv = nc.dram_tensor("Wv", [DIM, hd], F16, kind="ExternalInput")
    TrevT = nc.dram_tensor("TrevT", [D, RW], F16, kind="ExternalInput")
    VQT = nc.dram_tensor("VQT", [128, it, nh], F16, kind="ExternalInput")
    I128 = nc.dram_tensor("I128", [128, 128], F16, kind="ExternalInput")
    out = nc.dram_tensor("out", [npair, 2, 128, 512], F16, kind="ExternalOutput")
    zout = nc.dram_tensor("zout", [128, npair * 2 * it], F32, kind="ExternalOutput")

    with tile.TileContext(nc) as tc, ExitStack() as ctx:
        if bench_iters > 1:
            ctx.enter_context(
                tc.For_i(
                    0, bench_iters, 1,
                    hint_engines=(
                        mybir.EngineType.PE,
                        mybir.EngineType.DVE,
                        mybir.EngineType.Activation,
                        mybir.EngineType.SP,
                        mybir.EngineType.Pool,
                    ),
                    name="bench",
                )
            )
        # ---------------- persistent pools ----------------
        res = ctx.enter_context(tc.tile_pool(name="res", bufs=1))
        att = ctx.enter_context(tc.tile_pool(name="att", bufs=3))
        ptp_pool = ctx.enter_context(tc.tile_pool(name="ptp", bufs=1))
        ps_big = ctx.enter_context(tc.tile_pool(name="psB", bufs=3, space="PSUM"))
        ps_m = ctx.enter_context(tc.tile_pool(name="psM", bufs=2, space="PSUM"))

        trev_dup = res.tile([128, RW], F16, tag="trevdup")
        ident = res.tile([128, 128], F16, tag="ident")
        exp_bias = res.tile([128, 1], F32, tag="expbias")
        vqt = res.tile([128, it, nh], F16, tag="vqt")
        nc.gpsimd.memset(exp_bias[:], -C_SHIFT)

        x_r = res.tile([128, KC, seq], F16, tag="xr")
        wq_r = res.tile([128, KC, hd], F16, tag="wq")
        wk_r = res.tile([128, KC, hd], F16, tag="wk")
        wv_r = res.tile([128, KC, hd], F16, tag="wv")
        qT = [res.tile([128, seq], F16, tag=f"qT{m}", name=f"qT{m}") for m in range(npair)]
        kT = [res.tile([128, seq], F16, tag=f"kT{m}", name=f"kT{m}") for m in range(npair)]
        V64 = [res.tile([128, hd], F16, tag=f"V{j}", name=f"V{j}") for j in range(it)]
        z = res.tile([128, npair * 2 * it], F32, tag="z")
        # pair-combined P^T, double-buffered by pair parity: [k-part, hh, jb, q]
        ptp = [
            ptp_pool.tile([128, 2, it, seq], F16, tag=f"ptp{par}", name=f"ptp{par}")
            for par in range(min(2, npair))
        ]

        # ---------------- input loads (gating order) ----------------
        nc.scalar.dma_start(wq_r[:], Wq[:].rearrange("(a p) s -> p a s", p=128))
        nc.sync.dma_start(x_r[:], xT[:].rearrange("(a p) s -> p a s", p=128))
        nc.scalar.dma_start(wk_r[:], Wk[:].rearrange("(a p) s -> p a s", p=128))
        nc.sync.dma_start(trev_dup[0:64, :], TrevT[:])
        nc.sync.dma_start(trev_dup[64:128, :], TrevT[:])
        nc.scalar.dma_start(vqt[:], VQT[:])
        nc.sync.dma_start(ident[:], I128[:])
        nc.scalar.dma_start(wv_r[:], Wv[:].rearrange("(a p) s -> p a s", p=128))

        # ---------------- work units ----------------
        def proj_qk_group(m, which, sh):
            """One [128,512] projection group: q or k of pair m, seq half sh."""
            wr = wq_r if which == "q" else wk_r
            dest = qT if which == "q" else kT
            ps = ps_m.tile([128, 512], F32, tag="m512", bufs=2)
            for kc in range(KC):
                nc.tensor.matmul(
                    ps[:],
                    wr[:, kc, 128 * m: 128 * (m + 1)],
                    x_r[:, kc, 512 * sh: 512 * (sh + 1)],
                    start=(kc == 0),
                    stop=(kc == KC - 1),
                )
            nc.vector.tensor_copy(dest[m][:, 512 * sh: 512 * (sh + 1)], ps[:])

        def vproj_group(jt):
            ps = ps_m.tile([128, 512], F32, tag="m512", bufs=2)
            for kc in range(KC):
                nc.tensor.matmul(
                    ps[:, 0:hd],
                    x_r[:, kc, 128 * jt: 128 * (jt + 1)],
                    wv_r[:, kc, :],
                    start=(kc == 0),
                    stop=(kc == KC - 1),
                )
            nc.vector.tensor_copy(V64[jt][:], ps[:, 0:hd])

        exts = {}

        def qp_slice(m, tg, tt, hh):
            """QP matmuls + trimmed ext copy + edge fills for one (tile, head)."""
            t = GB * tg + tt
            h = 2 * m + hh
            lo, hi = _qp_range(t)
            base = 512 if t < GB else 0
            key = (h, tg)
            if key not in exts:
                exts[key] = att.tile(
                    [128, GB, EXTW], F16, tag="ext", name=f"ext{h}_{tg}", bufs=3
                )
            ext = exts[key]
            qps = ps_big.tile([128, 1024], F32, tag="big", bufs=3,
                              name=f"qp{h}_{t}")
            b64 = 64 * hh
            q_t = qT[m][b64: b64 + 64, 128 * t: 128 * (t + 1)]
            t_h = trev_dup[b64: b64 + 64, :]
            nc.tensor.matmul(qps[:, lo:512], q_t, t_h[:, lo:512],
                             start=True, stop=True)
            nc.tensor.matmul(qps[:, 512: hi + 1], q_t, t_h[:, 512: hi + 1],
                             start=True, stop=True)
            nc.vector.tensor_copy(
                ext[:, tt, 511 + lo - base: 512 + hi - base], qps[:, lo: hi + 1]
            )
            if t <= 3:
                nc.gpsimd.tensor_copy(ext[:, tt, 1023:1024], vqt[:, t, h: h + 1])
                rw = 511 - 128 * t
                if rw > 0:
                    nc.gpsimd.tensor_copy(
                        ext[:, tt, 1024: 1024 + rw],
                        ext[:, tt, 1023:1024].to_broadcast([128, rw]),
                    )
            else:
                lw = 128 * t - 385
                if lw > 0:
                    nc.gpsimd.tensor_copy(
                        ext[:, tt, 511 - lw: 511],
                        ext[:, tt, 511:512].to_broadcast([128, lw]),
                    )

        poss = {}

        def qp_skew(m, tg, hh):
            """Batched diagonal skew DMA for one head's 4-tile group (SWDGE)."""
            h = 2 * m + hh
            ext = exts.pop((h, tg))
            pos4 = att.tile([128, GB, seq], F16, tag="pos",
                            name=f"pos{h}_{tg}", bufs=3)
            poss[(h, tg)] = pos4
            extf = ext[:]
            diag = AP(
                tensor=extf.tensor,
                offset=extf.offset + 511,
                ap=[[GB * EXTW - 1, 128], [EXTW - 128, GB], [1, seq]],
            )
            nc.gpsimd.dma_start(pos4[:], diag)

        def emit_tile(m, t, pop_filler):
            """QK + pos-add + exp + transpose for both heads of (pair m, tile t)."""
            S = {}
            for hh in range(2):
                b64 = 64 * hh
                S[hh] = ps_big.tile([128, 1024], F32, tag="big", bufs=3,
                                    name=f"s{2*m+hh}_{t}")
                for half in range(2):
                    sl = slice(512 * half, 512 * (half + 1))
                    nc.tensor.matmul(
                        S[hh][:, sl],
                        qT[m][b64: b64 + 64, 128 * t: 128 * (t + 1)],
                        kT[m][b64: b64 + 64, sl],
                        start=True,
                        stop=False,
                    )
                pop_filler()
            for hh in range(2):
                h = 2 * m + hh
                pos4 = poss[(h, t // GB)]
                for half in range(2):
                    sl = slice(512 * half, 512 * (half + 1))
                    nc.tensor.matmul(
                        S[hh][:, sl],
                        ident[:],
                        pos4[:, t % GB, sl],
                        start=False,
                        stop=True,
                    )
                if t % GB == GB - 1:
                    poss.pop((h, t // GB), None)
            pop_filler()
            pt = att.tile([128, 2 * seq], F16, tag="pt", name=f"p{m}_{t}", bufs=3)
            for hh in range(2):
                zc = 16 * m + 8 * hh + t
                nc.scalar.activation(
                    pt[:, seq * hh: seq * (hh + 1)], S[hh][:],
                    mybir.ActivationFunctionType.Exp,
                    bias=exp_bias[:], scale=1.0,
                    accum_out=z[:, zc: zc + 1],
                )
            # per-head transposes on alternating HWDGE queues: finer DMA-pool
            # slices and two independent queues -> less head-of-line latency
            for hh in range(2):
                eng = nc.sync if hh == 0 else nc.scalar
                eng.dma_start_transpose(
                    out=ptp[m % 2][:, hh, :, 128 * t: 128 * (t + 1)],
                    in_=pt[:, seq * hh: seq * (hh + 1)],
                )
            pop_filler()

        def pv_chunk(m, blk):
            """PV for pair m, q-tile block blk (4 row-tiles): P^T chunks are
            the stationary operand, V the moving one, so the output lands in
            [q, e] orientation at full PE array utilization."""
            ps = ps_m.tile([128, 512], F32, tag="m512", bufs=2, name=f"pv{m}_{blk}")
            for sub in range(4):
                t = 4 * blk + sub
                for jb in range(it):
                    for hh in range(2):
                        h = 2 * m + hh
                        nc.tensor.matmul(
                            ps[:, 128 * sub + 64 * hh: 128 * sub + 64 * hh + 64],
                            ptp[m % 2][:, hh, jb, 128 * t: 128 * (t + 1)],
                            V64[jb][:, 64 * h: 64 * h + 64],
                            # one 2KB PSUM zero-region per bank: only the very
                            # first matmul may start, later ones write into
                            # pending-zero bytes (read-as-0 then accumulate)
                            start=(sub == 0 and jb == 0 and hh == 0),
                            stop=(sub == 3 and jb == it - 1 and hh == 1),
                            skip_group_check=True,
                        )
            ot = att.tile([128, 512], F16, tag="ot", bufs=2, name=f"ot{m}_{blk}")
            nc.scalar.copy(ot[:], ps[:])
            nc.scalar.dma_start(out[m][blk], ot[:])

        # ---------------- emission schedule ----------------
        def qp_units(m, tg):
            u = [(lambda m=m, tg=tg, tt=tt, hh=hh: qp_slice(m, tg, tt, hh))
                 for tt in range(GB) for hh in range(2)]
            u += [(lambda m=m, tg=tg, hh=hh: qp_skew(m, tg, hh)) for hh in range(2)]
            return u

        def proj_units(m):
            return [(lambda m=m, w=w, sh=sh: proj_qk_group(m, w, sh))
                    for w in ("q", "k") for sh in range(seq // 512)]

        if npair == 4 and seq == SEQ:
            # optimized interleave for the real config.
            # prologue: proj pair 0, then qp(0,0) zipped with proj pair 1 + v 0/1
            for u in proj_units(0):
                u()
            A = deque(qp_units(0, 0))
            Bq = deque(proj_units(1) + [lambda: vproj_group(0), lambda: vproj_group(1)])
            while A or Bq:
                if A:
                    A.popleft()()
                if Bq:
                    Bq.popleft()()
            # filler queue popped 4x per tile (pop index = (8m+t)*4 + k).
            # Ordered so every pos4 group lands >=2 tiles before its ident,
            # projections land before their consumers, and PV(m) pops only
            # after pair m's tiles are fully emitted (see need-by analysis).
            nopads = [None] * 8
            fill = deque(
                qp_units(0, 1)                                     # pops 0-9
                + qp_units(1, 0)                                   # 10-19
                + proj_units(2)                                    # 20-23
                + qp_units(1, 1)                                   # 24-33
                + qp_units(2, 0)                                   # 34-43
                + [(lambda jt=jt: vproj_group(jt)) for jt in range(2, it)]  # 44-49
                + [lambda: pv_chunk(0, 0), lambda: pv_chunk(0, 1)]  # 50-51
                + proj_units(3)                                    # 52-55
                + qp_units(2, 1)                                   # 56-65
                + qp_units(3, 0)                                   # 66-75
                + [lambda: pv_chunk(1, 0), lambda: pv_chunk(1, 1)]  # 76-77
                + qp_units(3, 1)                                   # 78-87
                + nopads + nopads                                  # 88-103
                + [lambda: pv_chunk(2, 0), lambda: pv_chunk(2, 1)]  # 104-105
            )

            def pop_filler():
                if fill:
                    u = fill.popleft()
                    if u is not None:
                        u()

            for m in range(npair):
                for t in range(it):
                    emit_tile(m, t, pop_filler)
            while fill:
                u = fill.popleft()
                if u is not None:
                    u()
            pv_chunk(3, 0)
            pv_chunk(3, 1)
        else:
            # simple order for small sim configs (correctness only)
            for m in range(npair):
                for u in proj_units(m):
                    u()
            for jt in range(it):
                vproj_group(jt)
            nop = lambda: None
            for m in range(npair):
                for tg in range(it // GB):
                    for u in qp_units(m, tg):
                        u()
                for t in range(it):
                    emit_tile(m, t, nop)
                pv_chunk(m, 0)
                pv_chunk(m, 1)

        nc.sync.dma_start(zout[:], z[:])

    nc.compile()
    return nc


def prep_inputs(x, Wq, Wkv, pos_emb):
    """Host-side shard prep: returns list of 8 per-core input dicts (fp16)."""
    x = np.asarray(x, dtype=np.float32)
    Wq = np.asarray(Wq, dtype=np.float32)
    Wkv = np.asarray(Wkv, dtype=np.float32)
    pos_emb = np.asarray(pos_emb, dtype=np.float32)

    Wq_s = (Wq * SCALE).astype(np.float32)
    trevT = np.ascontiguousarray(pos_emb[::-1].T[:, :RW])    # [64, 1024], col c = T[1024-c]
    trevT16 = trevT.astype(np.float16)
    ident = np.eye(128, dtype=np.float16)
    it = SEQ // 128

    in_maps = []
    for c in range(NC):
        b, hg = c // 2, c % 2
        hs = slice(HD * hg, HD * (hg + 1))
        wq_c = np.ascontiguousarray(Wq_s[:, hs])
        # VQ: per-head derived value so that vq[i, h] = q_h(i) . T[0] (dist -512)
        wstar = np.einsum(
            "dhe,e->dh", wq_c.reshape(DIM, NH, D), pos_emb[0].astype(np.float32)
        )
        vq = (x[b] @ wstar).astype(np.float16)               # [seq, nh]
        vqt = np.ascontiguousarray(vq.reshape(it, 128, NH).transpose(1, 0, 2))
        in_maps.append(
            {
                "xT": np.ascontiguousarray(x[b].T).astype(np.float16),
                "Wq": wq_c.astype(np.float16),
                "Wk": np.ascontiguousarray(Wkv[:, hs]).astype(np.float16),
                "Wv": np.ascontiguousarray(Wkv[:, DIM:][:, hs]).astype(np.float16),
                "TrevT": trevT16,
                "VQT": vqt,
                "I128": ident,
            }
        )
    return in_maps


def assemble(results):
    """results: list of 8 out maps ({'out': [4,128,1024] f16, 'zout': [128,64] f32})
    -> [4,16,1024,64] f32.  Normalization (divide by Z) happens here."""
    it = SEQ // 128
    out = np.empty((B, N_HEADS_TOT, SEQ, D), dtype=np.float32)
    for c in range(NC):
        b, hg = c // 2, c % 2
        o = np.asarray(results[c]["out"], dtype=np.float32)   # [4, 2, 128, 512]
        zf = np.asarray(results[c]["zout"], dtype=np.float32) # [128, 64]
        z = zf.reshape(128, NH // 2, 2, it)
        # o[m, qb, r, 128*sub + 64*hh + e] = unnorm out(head 2m+hh, i=128*(4qb+sub)+r, e)
        o5 = o.reshape(NH // 2, 2, 128, 4, 2, D)              # [m, qb, r, sub, hh, e]
        for m in range(NH // 2):
            for hh in range(2):
                h = NH * hg + 2 * m + hh
                un = o5[m, :, :, :, hh, :].transpose(0, 2, 1, 3).reshape(SEQ, D)
                zi = z[:, m, hh, :].T.reshape(SEQ)            # [1024]
                out[b, h] = un / zi[:, None]
    return out


def kernel(x, Wq, Wkv, pos_emb, trace=False, trace_kwargs=None, bench_iters=1, ablate=()):
    key = ("nc", bench_iters, tuple(sorted(ablate)))
    if key not in _cached:
        _cached[key] = build_nc(bench_iters=bench_iters, ablate=ablate)
    nc = _cached[key]
    in_maps = prep_inputs(x, Wq, Wkv, pos_emb)
    res = run_bass_kernel_spmd(
        nc, in_maps, list(range(NC)), trace=trace, **(trace_kwargs or {})
    )
    out = assemble(res.results)
    if trace:
        _cached["last_result"] = res
    return out


if __name__ == "__main__":
    rng = np.random.default_rng(0)
    x = rng.standard_normal((B, SEQ, DIM), dtype=np.float32)
    Wq = (rng.standard_normal((DIM, 1024), dtype=np.float32) * DIM ** -0.5)
    Wkv = (rng.standard_normal((DIM, 2048), dtype=np.float32) * DIM ** -0.5)
    pos_emb = rng.standard_normal((TABLE, D), dtype=np.float32)
    out = kernel(x, Wq, Wkv, pos_emb)
    print("out shape", out.shape, "finite:", np.isfinite(out).all())
